# revision 12
# baseline (speedup 1.0000x reference)
"""Multi-head attention (B=4, S=2048, H=8 heads, d_head=16) on 8 trn2 cores.

Sharding: one head per core. Per head h, per batch b (nb = ceil(seq_len/128)
k-tiles), a transposed-scores dataflow tuned to the TimelineSim cost model:

  S^T[k, q] (PSUM, f32) via fp8e4m3 DoubleRow matmuls (0.5 cyc/row):
      Q,K prescaled by 4 host-side and split hi/mid/lo in e4m3; the 6
      significant cross terms (96 contraction rows) + 1 offset/mask row are
      stacked as 2 DoubleRow blocks of 49 partitions. Stored logits are
      16*s + 128 (valid) where s = q.k; masked k rows get 16*s - 1920.
  P^T = unnormalized softmax weights, split by q-halves across 2 engines
      (GPSIMD cannot read PSUM, so it only assists with DMA dispatch):
      ACT:  exact exp(0.25*in + bias), bf16 out   [cols 0:1024 of each unit]
      DVE:  bf16-domain Schraudolph bit-trick     [cols 1024:2048]
            bitcast16(uint16(max(in*C1B, 0)))
      The exponent bias rides the matmul offset row; the Schraudolph center
      correction rides the ACT bias so both paths share one global factor.
  out[q, 34] += matmul(lhsT=P^T[128k, 128q], rhs=vo[128k, 34])
      P^T is the *stationary* operand (weight loads are free), vo streams
      34 bf16 rows: [v_hi(16) | 1 | v_lo(16) | 0]. 16 q-tile accumulators
      per batch live 8-per-PSUM-bank via the pending-zero start trick.

Rows 16/33 of the output hold the softmax denominator; host divides.
"""

import ml_dtypes
import numpy as np

import concourse.bass as bass
import concourse.tile as tile
from concourse import bacc, mybir
from concourse.bass_utils import run_bass_kernel_spmd

B = 4
S = 2048
H = 8
DH = 16
KT_TILE = 128
F32 = mybir.dt.float32
F32R = mybir.dt.float32r
BF16 = mybir.dt.bfloat16
FP8 = mybir.dt.float8e4
U16 = mybir.dt.uint16
E4 = ml_dtypes.float8_e4m3
BFLOAT16 = ml_dtypes.bfloat16

# stored logits are 16*s + 128 (s = raw q.k dot); exp arg is 4*s.
# Schraudolph runs in the bf16 bit domain (uint16 write; the max(.,0) clamp
# keeps bits < 32768 so the sign bit never sets).
C1B = float(0.25 * (2.0 ** 7) / np.log(2.0))  # stored -> bf16 exponent bits
CENTER = 1.02750  # Schraudolph sawtooth centering, folded into ACT bias
_F_LOG2 = (128.0 * C1B) / 2.0 ** 7 - 127.0
ACT_BIAS = float(-32.0 + _F_LOG2 * np.log(2.0) + np.log(CENTER))
W_VALID = 16.0
W_MASK = -240.0

_cache = {}


def _build(nbs):
    nb_total = sum(nbs)

    nc = bacc.Bacc(
        "TRN2",
        target_bir_lowering=False,
        debug=False,
        num_devices=8,
    )

    qT_d = nc.dram_tensor("qT", [B, 49, 2, S], FP8, kind="ExternalInput").ap()
    kT_d = nc.dram_tensor(
        "kT", [49, 2, nb_total * 128], FP8, kind="ExternalInput"
    ).ap()
    vo_d = nc.dram_tensor("vo", [128, nb_total * 34], BF16, kind="ExternalInput").ap()
    out_d = nc.dram_tensor("out", [B, 128, 16, 34], F32, kind="ExternalOutput").ap()

    mult = mybir.AluOpType.mult
    amax = mybir.AluOpType.max
    DR = mybir.MatmulPerfMode.DoubleRow

    with tile.TileContext(nc) as tc:
        with (
            tc.tile_pool(name="const", bufs=1) as const,
            tc.tile_pool(name="st", bufs=3, space="PSUM") as stpool,
            tc.tile_pool(name="pt", bufs=8) as ptpool,
            tc.tile_pool(name="ot", bufs=2, space="PSUM") as otpool,
            tc.tile_pool(name="ob", bufs=3) as obpool,
        ):
            q_tiles = []
            for b in range(B):
                qt = const.tile([49, 2, S], FP8, tag=f"qT{b}")
                q_tiles.append(qt)
            kT_t = const.tile([49, 2, nb_total * 128], FP8, tag="kT")
            vo_t = const.tile([128, nb_total * 34], BF16, tag="vo")

            # Critical-path DMAs first: k-tile 0 + vo of batch 0 on the sync
            # HWDGE ring; qT batch 0 on the scalar ring (dispatched before
            # the ACT warm-up). Remaining bulk splits across both rings.
            nc.sync.dma_start(kT_t[:, :, 0:128], kT_d[:, :, 0:128])
            nc.scalar.dma_start(q_tiles[0][:, :, 0:512], qT_d[0][:, :, 0:512])
            nc.sync.dma_start(
                kT_t[:, :, 128:nbs[0] * 128], kT_d[:, :, 128:nbs[0] * 128]
            )
            nc.scalar.dma_start(q_tiles[0][:, :, 512:1024], qT_d[0][:, :, 512:1024])
            nc.scalar.dma_start(q_tiles[0][:, :, 1024:2048], qT_d[0][:, :, 1024:2048])
            nc.sync.dma_start(
                vo_t[:, 0:nbs[0] * 34], vo_d[:, 0:nbs[0] * 34]
            )
            # ACT exp table prefetch while DMAs run; bias vector for the
            # exact-exp path (bass needs an AP, not a float const).
            warm = const.tile([1, 1], F32, tag="warm")
            nc.vector.memset(warm[:], 0.0)
            bias_t = const.tile([128, 1], F32, tag="bias")
            nc.vector.memset(bias_t[:], ACT_BIAS)
            nc.scalar.activation(
                warm[:], warm[:], mybir.ActivationFunctionType.Exp,
                bias=bias_t[0:1, :],
            )
            # PE clock-gate warm-up on zeroed data; lands in an st-pool slot
            # later cleared by the first real matmul's start=True.
            pewarm = const.tile([128, 64], BF16, tag="pewarm")
            nc.vector.memset(pewarm[:], 0.0)
            st_w = stpool.tile([128, 1024], F32, tag="st", name="st_w")
            for j in range(2):
                nc.tensor.matmul(
                    st_w[0:64, 64 * j:64 * (j + 1)],
                    pewarm[:, 0:64],
                    pewarm[:],
                    start=True,
                    stop=True,
                )
            for b in range(1, B):
                off = sum(nbs[:b])
                nb = nbs[b]
                nc.sync.dma_start(q_tiles[b][:], qT_d[b])
                nc.sync.dma_start(
                    kT_t[:, :, off * 128:(off + nb) * 128],
                    kT_d[:, :, off * 128:(off + nb) * 128],
                )
                nc.sync.dma_start(
                    vo_t[:, off * 34:(off + nb) * 34],
                    vo_d[:, off * 34:(off + nb) * 34],
                )

            # Half-units: one per (batch, k-tile, 1024-wide q chunk).
            hus = []
            for b in range(B):
                for kt in range(nbs[b]):
                    off = sum(nbs[:b])
                    for half in range(2):
                        hus.append(
                            (b, kt, half, off + kt, kt == 0, kt == nbs[b] - 1)
                        )

            pts = {}
            ots = {}
            eng_load = [0.0, 0.0]  # ACT, DVE accumulated ns

            def emit_st(u):
                b, kt, half, t, _, _ = hus[u]
                st = stpool.tile([128, 1024], F32, tag="st", name="st")
                for j in range(2):
                    qs = 1024 * half + 512 * j
                    nc.tensor.matmul(
                        st[:, 512 * j:512 * (j + 1)],
                        kT_t[:, :, t * 128:(t + 1) * 128],
                        q_tiles[b][:, :, qs:qs + 512],
                        start=True,
                        stop=True,
                        perf_mode=DR,
                    )
                pt = ptpool.tile([128, 1024], BF16, tag="pt", name="pt")
                if eng_load[0] + 1070.0 <= eng_load[1] + 1237.0:
                    eng_load[0] += 1070.0
                    nc.scalar.activation(
                        pt[:],
                        st[:],
                        mybir.ActivationFunctionType.Exp,
                        bias=bias_t[:],
                        scale=0.25,
                    )
                else:
                    eng_load[1] += 1237.0
                    nc.vector.tensor_scalar(
                        pt[:].bitcast(U16), st[:], C1B, 0.0, mult, amax
                    )
                pts[u] = pt

            def emit_av(u):
                b, kt, half, t, first, last = hus[u]
                if first:
                    ots[(b, half)] = otpool.tile([128, 8, 64], F32, tag="ot", name="ot")
                ot = ots[(b, half)]
                pt = pts.pop(u)
                for jj in range(8):
                    nc.tensor.matmul(
                        ot[:, jj, 0:34],
                        pt[:, 128 * jj:128 * (jj + 1)],
                        vo_t[:, t * 34:(t + 1) * 34],
                        start=(first and jj == 0),
                        stop=(last and jj == 7),
                        skip_group_check=True,
                    )
                if last:
                    ob = obpool.tile([128, 8, 34], F32, tag="ob", name="ob")
                    if eng_load[0] + 444.0 <= eng_load[1] + 453.0:
                        eng_load[0] += 444.0
                        nc.scalar.copy(ob[:], ot[:, :, 0:34])
                    else:
                        eng_load[1] += 453.0
                        nc.vector.tensor_copy(ob[:], ot[:, :, 0:34])
                    nc.gpsimd.dma_start(
                        out_d[b][:, 8 * half:8 * half + 8, :], ob[:]
                    )

            DIST = 5
            for u in range(len(hus)):
                emit_st(u)
                if u >= DIST:
                    emit_av(u - DIST)
            for u in range(len(hus) - DIST, len(hus)):
                emit_av(u)

    nc.compile()
    return nc


def kernel(key_and_value, query, seq_len):
    key_and_value = np.asarray(key_and_value, dtype=np.float32)
    query = np.asarray(query, dtype=np.float32)
    sl = np.asarray(seq_len).reshape(-1).astype(np.int64)

    nbs = tuple(int(-(-int(s) // KT_TILE)) for s in sl)
    nb_total = sum(nbs)

    if nbs not in _cache:
        _cache[nbs] = _build(nbs)
    nc = _cache[nbs]

    k_all = key_and_value[:, :, :128]
    v_all = key_and_value[:, :, 128:]

    def split3(x):
        hi = x.astype(E4)
        r = x - hi.astype(np.float32)
        mid = r.astype(E4)
        lo = (r - mid.astype(np.float32)).astype(E4)
        return hi, mid, lo

    q4T = np.ascontiguousarray((4.0 * query).transpose(0, 2, 1))  # [B,128,S]
    k4T = np.ascontiguousarray((4.0 * k_all).transpose(0, 2, 1))  # [B,128,S]
    qh_a, qm_a, ql_a = split3(q4T)
    kh_a, km_a, kl_a = split3(k4T)

    in_maps = []
    for h in range(H):
        c0 = h * DH
        qT = np.zeros((B, 49, 2, S), dtype=E4)
        for i, part in enumerate([qh_a, qh_a, qm_a]):
            qT[:, i * DH:(i + 1) * DH, 0, :] = part[:, c0:c0 + DH]
        for i, part in enumerate([qh_a, ql_a, qm_a]):
            qT[:, i * DH:(i + 1) * DH, 1, :] = part[:, c0:c0 + DH]
        qT[:, 48, 0, :] = np.float32(8.0)

        kT = np.zeros((49, 2, nb_total * 128), dtype=E4)
        vo = np.zeros((128, nb_total * 34), dtype=BFLOAT16)
        for b in range(B):
            off = sum(nbs[:b])
            nrow = nbs[b] * 128
            cs = slice(off * 128, off * 128 + nrow)
            for i, part in enumerate([kh_a, km_a, kh_a]):
                kT[i * DH:(i + 1) * DH, 0, cs] = part[b, c0:c0 + DH, :nrow]
            for i, part in enumerate([kl_a, kh_a, km_a]):
                kT[i * DH:(i + 1) * DH, 1, cs] = part[b, c0:c0 + DH, :nrow]
            wk = np.where(
                np.arange(nrow) < sl[b], np.float32(W_VALID), np.float32(W_MASK)
            )
            kT[48, 0, cs] = wk.astype(E4)

            vv = v_all[b, :nrow, c0:c0 + DH].astype(np.float32)
            vhi = vv.astype(BFLOAT16).astype(np.float32)
            vlo = vv - vhi
            vo_b = np.zeros((nbs[b], 128, 34), dtype=np.float32)
            vo_b[:, :, 0:16] = vhi.reshape(nbs[b], 128, 16)
            vo_b[:, :, 16] = 1.0
            vo_b[:, :, 17:33] = vlo.reshape(nbs[b], 128, 16)
            vo[:, off * 34:(off + nbs[b]) * 34] = (
                vo_b.transpose(1, 0, 2).reshape(128, nbs[b] * 34).astype(BFLOAT16)
            )

        in_maps.append({
            "qT": np.ascontiguousarray(qT),
            "kT": np.ascontiguousarray(kT),
            "vo": np.ascontiguousarray(vo),
        })

    import os

    trace = bool(os.environ.get("ATTN_TRACE"))
    kw = {}
    if trace:
        kw = dict(
            trace=True,
            tmpdir=os.environ.get("ATTN_TRACE_DIR") or None,
            trace_cores=[0],
        )
    res = run_bass_kernel_spmd(nc, in_maps, core_ids=list(range(H)), **kw)
    if trace and res.exec_time_ns is not None:
        print(f"HW exec time: {res.exec_time_ns} ns")
        kernel.last_exec_time_ns = res.exec_time_ns

    out = np.empty((B, S, H * DH), dtype=np.float32)
    for h in range(H):
        o = res.results[h]["out"]  # [B, 128, 16, 34]
        num = o[:, :, :, 0:16] + o[:, :, :, 17:33]
        den = o[:, :, :, 16:17] + o[:, :, :, 33:34]
        r = num / den  # [B, 128p, 16j, 16c]
        out[:, :, h * DH:(h + 1) * DH] = (
            r.transpose(0, 2, 1, 3).reshape(B, S, DH)
        )
    return out


# revision 13
# speedup vs baseline: 1.0192x; 1.0192x over previous
"""Multi-head attention (B=4, S=2048, H=8 heads, d_head=16) on 8 trn2 cores.

Sharding: one head per core. Per head h, per batch b (nb = ceil(seq_len/128)
k-tiles), a transposed-scores dataflow tuned to the TimelineSim cost model:

  S^T[k, q] (PSUM, f32) via fp8e4m3 DoubleRow matmuls (0.5 cyc/row):
      Q,K prescaled by 4 host-side and split hi/mid/lo in e4m3; the 6
      significant cross terms (96 contraction rows) + 1 offset/mask row are
      stacked as 2 DoubleRow blocks of 49 partitions. Stored logits are
      16*s + 128 (valid) where s = q.k; masked k rows get 16*s - 1920.
  P^T = unnormalized softmax weights, split by q-halves across 2 engines
      (GPSIMD cannot read PSUM, so it only assists with DMA dispatch):
      ACT:  exact exp(0.25*in + bias), bf16 out   [cols 0:1024 of each unit]
      DVE:  bf16-domain Schraudolph bit-trick     [cols 1024:2048]
            bitcast16(uint16(max(in*C1B, 0)))
      The exponent bias rides the matmul offset row; the Schraudolph center
      correction rides the ACT bias so both paths share one global factor.
  out[q, 34] += matmul(lhsT=P^T[128k, 128q], rhs=vo[128k, 34])
      P^T is the *stationary* operand (weight loads are free), vo streams
      34 bf16 rows: [v_hi(16) | 1 | v_lo(16) | 0]. 16 q-tile accumulators
      per batch live 8-per-PSUM-bank via the pending-zero start trick.

Rows 16/33 of the output hold the softmax denominator; host divides.
"""

import ml_dtypes
import numpy as np

import concourse.bass as bass
import concourse.tile as tile
from concourse import bacc, mybir
from concourse.bass_utils import run_bass_kernel_spmd

B = 4
S = 2048
H = 8
DH = 16
KT_TILE = 128
F32 = mybir.dt.float32
F32R = mybir.dt.float32r
BF16 = mybir.dt.bfloat16
FP8 = mybir.dt.float8e4
U16 = mybir.dt.uint16
E4 = ml_dtypes.float8_e4m3
BFLOAT16 = ml_dtypes.bfloat16

# stored logits are 16*s + 128 (s = raw q.k dot); exp arg is 4*s.
# Schraudolph runs in the bf16 bit domain (uint16 write; the max(.,0) clamp
# keeps bits < 32768 so the sign bit never sets).
C1B = float(0.25 * (2.0 ** 7) / np.log(2.0))  # stored -> bf16 exponent bits
CENTER = 1.02750  # Schraudolph sawtooth centering, folded into ACT bias
_F_LOG2 = (128.0 * C1B) / 2.0 ** 7 - 127.0
ACT_BIAS = float(-32.0 + _F_LOG2 * np.log(2.0) + np.log(CENTER))
W_VALID = 16.0
W_MASK = -240.0

_cache = {}


def _build(nbs):
    nb_total = sum(nbs)

    nc = bacc.Bacc(
        "TRN2",
        target_bir_lowering=False,
        debug=False,
        num_devices=8,
    )

    qT_d = nc.dram_tensor("qT", [B, 49, 2, S], FP8, kind="ExternalInput").ap()
    kT_d = nc.dram_tensor(
        "kT", [49, 2, nb_total * 128], FP8, kind="ExternalInput"
    ).ap()
    vo_d = nc.dram_tensor("vo", [128, nb_total * 34], BF16, kind="ExternalInput").ap()
    out_d = nc.dram_tensor("out", [B, 128, 16, 34], F32, kind="ExternalOutput").ap()

    mult = mybir.AluOpType.mult
    amax = mybir.AluOpType.max
    DR = mybir.MatmulPerfMode.DoubleRow

    with tile.TileContext(nc) as tc:
        with (
            tc.tile_pool(name="const", bufs=1) as const,
            tc.tile_pool(name="st", bufs=3, space="PSUM") as stpool,
            tc.tile_pool(name="pt", bufs=8) as ptpool,
            tc.tile_pool(name="ot", bufs=2, space="PSUM") as otpool,
            tc.tile_pool(name="ob", bufs=3) as obpool,
        ):
            q_tiles = []
            for b in range(B):
                qt = const.tile([49, 2, S], FP8, tag=f"qT{b}")
                q_tiles.append(qt)
            kT_t = const.tile([49, 2, nb_total * 128], FP8, tag="kT")
            vo_t = const.tile([128, nb_total * 34], BF16, tag="vo")

            # Critical-path DMAs first: k-tile 0 + vo of batch 0 on the sync
            # HWDGE ring; qT batch 0 on the scalar ring (dispatched before
            # the ACT warm-up). Remaining bulk splits across both rings.
            nc.sync.dma_start(kT_t[:, :, 0:128], kT_d[:, :, 0:128])
            nc.scalar.dma_start(q_tiles[0][:, :, 0:512], qT_d[0][:, :, 0:512])
            nc.sync.dma_start(
                kT_t[:, :, 128:nbs[0] * 128], kT_d[:, :, 128:nbs[0] * 128]
            )
            nc.scalar.dma_start(q_tiles[0][:, :, 512:1024], qT_d[0][:, :, 512:1024])
            nc.scalar.dma_start(q_tiles[0][:, :, 1024:2048], qT_d[0][:, :, 1024:2048])
            nc.sync.dma_start(
                vo_t[:, 0:nbs[0] * 34], vo_d[:, 0:nbs[0] * 34]
            )
            # ACT exp table prefetch while DMAs run; bias vector for the
            # exact-exp path (bass needs an AP, not a float const).
            warm = const.tile([1, 1], F32, tag="warm")
            nc.vector.memset(warm[:], 0.0)
            bias_t = const.tile([128, 1], F32, tag="bias")
            nc.vector.memset(bias_t[:], ACT_BIAS)
            nc.scalar.activation(
                warm[:], warm[:], mybir.ActivationFunctionType.Exp,
                bias=bias_t[0:1, :],
            )
            # PE clock-gate warm-up on zeroed data; lands in an st-pool slot
            # later cleared by the first real matmul's start=True.
            pewarm = const.tile([128, 64], BF16, tag="pewarm")
            nc.vector.memset(pewarm[:], 0.0)
            st_w = stpool.tile([128, 1024], F32, tag="st", name="st_w")
            for j in range(2):
                nc.tensor.matmul(
                    st_w[0:64, 64 * j:64 * (j + 1)],
                    pewarm[:, 0:64],
                    pewarm[:],
                    start=True,
                    stop=True,
                )
            for b in range(1, B):
                off = sum(nbs[:b])
                nb = nbs[b]
                nc.sync.dma_start(q_tiles[b][:], qT_d[b])
                nc.sync.dma_start(
                    kT_t[:, :, off * 128:(off + nb) * 128],
                    kT_d[:, :, off * 128:(off + nb) * 128],
                )
                nc.sync.dma_start(
                    vo_t[:, off * 34:(off + nb) * 34],
                    vo_d[:, off * 34:(off + nb) * 34],
                )

            # Half-units: one per (batch, k-tile, 1024-wide q chunk).
            hus = []
            for b in range(B):
                for kt in range(nbs[b]):
                    off = sum(nbs[:b])
                    for half in range(2):
                        hus.append(
                            (b, kt, half, off + kt, kt == 0, kt == nbs[b] - 1)
                        )

            pts = {}
            ots = {}
            eng_load = [0.0, 0.0]  # ACT, DVE accumulated ns

            def emit_st(u):
                b, kt, half, t, _, _ = hus[u]
                st = stpool.tile([128, 1024], F32, tag="st", name="st")
                for j in range(2):
                    qs = 1024 * half + 512 * j
                    nc.tensor.matmul(
                        st[:, 512 * j:512 * (j + 1)],
                        kT_t[:, :, t * 128:(t + 1) * 128],
                        q_tiles[b][:, :, qs:qs + 512],
                        start=True,
                        stop=True,
                        perf_mode=DR,
                    )
                pt = ptpool.tile([128, 1024], BF16, tag="pt", name="pt")
                if eng_load[0] + 1070.0 <= eng_load[1] + 1237.0:
                    eng_load[0] += 1070.0
                    nc.scalar.activation(
                        pt[:],
                        st[:],
                        mybir.ActivationFunctionType.Exp,
                        bias=bias_t[:],
                        scale=0.25,
                    )
                else:
                    eng_load[1] += 1237.0
                    nc.vector.tensor_scalar(
                        pt[:].bitcast(U16), st[:], C1B, 0.0, mult, amax
                    )
                pts[u] = pt

            def emit_av(u):
                b, kt, half, t, first, last = hus[u]
                if first:
                    ots[(b, half)] = otpool.tile([128, 8, 64], F32, tag="ot", name="ot")
                ot = ots[(b, half)]
                pt = pts.pop(u)
                for jj in range(8):
                    nc.tensor.matmul(
                        ot[:, jj, 0:34],
                        pt[:, 128 * jj:128 * (jj + 1)],
                        vo_t[:, t * 34:(t + 1) * 34],
                        start=(first and jj == 0),
                        stop=(last and jj == 7),
                        skip_group_check=True,
                    )
                if last:
                    ob = obpool.tile([128, 8, 34], F32, tag="ob", name="ob")
                    if eng_load[0] + 444.0 <= eng_load[1] + 453.0:
                        eng_load[0] += 444.0
                        nc.scalar.copy(ob[:], ot[:, :, 0:34])
                    else:
                        eng_load[1] += 453.0
                        nc.vector.tensor_copy(ob[:], ot[:, :, 0:34])
                    nc.sync.dma_start(
                        out_d[b][:, 8 * half:8 * half + 8, :], ob[:]
                    )

            DIST = 5
            for u in range(len(hus)):
                emit_st(u)
                if u >= DIST:
                    emit_av(u - DIST)
            for u in range(len(hus) - DIST, len(hus)):
                emit_av(u)

    nc.compile()
    return nc


def kernel(key_and_value, query, seq_len):
    key_and_value = np.asarray(key_and_value, dtype=np.float32)
    query = np.asarray(query, dtype=np.float32)
    sl = np.asarray(seq_len).reshape(-1).astype(np.int64)

    nbs = tuple(int(-(-int(s) // KT_TILE)) for s in sl)
    nb_total = sum(nbs)

    if nbs not in _cache:
        _cache[nbs] = _build(nbs)
    nc = _cache[nbs]

    k_all = key_and_value[:, :, :128]
    v_all = key_and_value[:, :, 128:]

    def split3(x):
        hi = x.astype(E4)
        r = x - hi.astype(np.float32)
        mid = r.astype(E4)
        lo = (r - mid.astype(np.float32)).astype(E4)
        return hi, mid, lo

    q4T = np.ascontiguousarray((4.0 * query).transpose(0, 2, 1))  # [B,128,S]
    k4T = np.ascontiguousarray((4.0 * k_all).transpose(0, 2, 1))  # [B,128,S]
    qh_a, qm_a, ql_a = split3(q4T)
    kh_a, km_a, kl_a = split3(k4T)

    in_maps = []
    for h in range(H):
        c0 = h * DH
        qT = np.zeros((B, 49, 2, S), dtype=E4)
        for i, part in enumerate([qh_a, qh_a, qm_a]):
            qT[:, i * DH:(i + 1) * DH, 0, :] = part[:, c0:c0 + DH]
        for i, part in enumerate([qh_a, ql_a, qm_a]):
            qT[:, i * DH:(i + 1) * DH, 1, :] = part[:, c0:c0 + DH]
        qT[:, 48, 0, :] = np.float32(8.0)

        kT = np.zeros((49, 2, nb_total * 128), dtype=E4)
        vo = np.zeros((128, nb_total * 34), dtype=BFLOAT16)
        for b in range(B):
            off = sum(nbs[:b])
            nrow = nbs[b] * 128
            cs = slice(off * 128, off * 128 + nrow)
            for i, part in enumerate([kh_a, km_a, kh_a]):
                kT[i * DH:(i + 1) * DH, 0, cs] = part[b, c0:c0 + DH, :nrow]
            for i, part in enumerate([kl_a, kh_a, km_a]):
                kT[i * DH:(i + 1) * DH, 1, cs] = part[b, c0:c0 + DH, :nrow]
            wk = np.where(
                np.arange(nrow) < sl[b], np.float32(W_VALID), np.float32(W_MASK)
            )
            kT[48, 0, cs] = wk.astype(E4)

            vv = v_all[b, :nrow, c0:c0 + DH].astype(np.float32)
            vhi = vv.astype(BFLOAT16).astype(np.float32)
            vlo = vv - vhi
            vo_b = np.zeros((nbs[b], 128, 34), dtype=np.float32)
            vo_b[:, :, 0:16] = vhi.reshape(nbs[b], 128, 16)
            vo_b[:, :, 16] = 1.0
            vo_b[:, :, 17:33] = vlo.reshape(nbs[b], 128, 16)
            vo[:, off * 34:(off + nbs[b]) * 34] = (
                vo_b.transpose(1, 0, 2).reshape(128, nbs[b] * 34).astype(BFLOAT16)
            )

        in_maps.append({
            "qT": np.ascontiguousarray(qT),
            "kT": np.ascontiguousarray(kT),
            "vo": np.ascontiguousarray(vo),
        })

    import os

    trace = bool(os.environ.get("ATTN_TRACE"))
    kw = {}
    if trace:
        kw = dict(
            trace=True,
            tmpdir=os.environ.get("ATTN_TRACE_DIR") or None,
            trace_cores=[0],
        )
    res = run_bass_kernel_spmd(nc, in_maps, core_ids=list(range(H)), **kw)
    if trace and res.exec_time_ns is not None:
        print(f"HW exec time: {res.exec_time_ns} ns")
        kernel.last_exec_time_ns = res.exec_time_ns

    out = np.empty((B, S, H * DH), dtype=np.float32)
    for h in range(H):
        o = res.results[h]["out"]  # [B, 128, 16, 34]
        num = o[:, :, :, 0:16] + o[:, :, :, 17:33]
        den = o[:, :, :, 16:17] + o[:, :, :, 33:34]
        r = num / den  # [B, 128p, 16j, 16c]
        out[:, :, h * DH:(h + 1) * DH] = (
            r.transpose(0, 2, 1, 3).reshape(B, S, DH)
        )
    return out
